# revision 1
# baseline (speedup 1.0000x reference)
# DMPNN encoder layer on 8 Trainium2 NeuronCores (Bass/Tile).
#
# Distribution: data-parallel over bonds (16384/core) and atoms (8192/core).
# Gather-sum rounds use windowed dma_gather (int16 indices -> 4 windows of
# 32768 table rows) + dma_scatter_add (CCE add performs the sum over incoming
# bonds; j-major chunking keeps dest indices unique within a scatter call).
# Message tables are bf16 [*, 384] rows; full tables are replicated across
# cores via AllGather between rounds. The reference's first-iteration
# h_message is dead, so only one W_h matmul is needed:
#   m1 = gsum(relu(f_ini @ W_i)); m2 = gsum(m1)
#   h2 = relu(f_ini @ W_i + m2 @ W_h)
#   msgs = gsum_atoms(h2); atoms_h = relu([atom_f, msgs] @ W_o + b_o)
#   out = [segment_mean(atoms_h), global_features]
import numpy as np

B = 131072        # bonds
A = 65536         # atoms
APM = 32          # atoms per molecule
D = 300           # hidden
DP = 384          # padded row (768B bf16, %256B for dma_gather)
F = 147           # bond input features
AF = 133          # atom features
NC = 8            # cores
BS = B // NC      # bond shard
AS = A // NC      # atom shard
MS = 2048 // NC   # molecules per core
WIN = 32768       # rows per window tensor
QS = 4096         # quarter-shard rows (window interleave unit)
NW = B // WIN     # 4 windows
SL = 512          # bonds per matmul slab

_CACHE = {}
LAST_RESULTS = None


def _pad128(n):
    return (n + 127) & ~127


def _wrap_idx(flat):
    """[L] -> [128, L/16] int16: idx i at (partition i%16, col i//16),
    replicated across the 8 gpsimd core groups."""
    L = len(flat)
    w = flat.reshape(L // 16, 16).T.astype(np.int16)
    return np.tile(w, (8, 1))


def _plan_round(src, trash, n_half):
    """src: [n_dest, 4] global source rows. Returns (chunks, counts) where
    chunks[k] = per-window (gather_local_idx, dest_idx) lists for chunk
    k = (j, half); counts[k][w] = real entries in that window."""
    n_dest = src.shape[0]
    half = n_dest // n_half
    chunks, counts = [], []
    for j in range(4):
        for h in range(n_half):
            lo, hi = h * half, (h + 1) * half
            s = src[lo:hi, j]
            dest = np.arange(lo, hi, dtype=np.int32)
            # window w holds quarter w of every core's shard (AG#w output):
            # global row g -> window (g%BS)//QS, local (g//BS)*QS + g%QS
            w = (s % BS) // QS
            loc = (s // BS) * QS + (s % QS)
            per_w = []
            cnt = []
            for wi in range(NW):
                sel = np.nonzero(w == wi)[0]
                per_w.append((loc[sel], dest[sel]))
                cnt.append(len(sel))
            chunks.append(per_w)
            counts.append(cnt)
    return chunks, np.array(counts, np.int64)


def _pack_round(all_core_plans, trash):
    """Uniform widths across cores: widths[k][w] = pad128(max count).
    Returns (widths, GL, per-core (g_idx, s_idx) arrays [nchunk*128, GL/16])."""
    counts = np.stack([c for (_, c) in all_core_plans])       # [NC, nk, NW]
    widths = np.array([[_pad128(int(x)) for x in row]
                       for row in counts.max(axis=0)], np.int64)
    GL = _pad128(int(widths.sum(axis=1).max()))
    packed = []
    for chunks, _ in all_core_plans:
        gs, ss = [], []
        for k, per_w in enumerate(chunks):
            gp = np.zeros(GL, np.int32)           # pad = row 0 (valid, cheap)
            sp = np.full(GL, trash, np.int32)     # pad dest = trash row
            off = 0
            for wi in range(NW):
                loc, dest = per_w[wi]
                gp[off:off + len(loc)] = loc
                sp[off:off + len(dest)] = dest
                off += int(widths[k][wi])
            gs.append(_wrap_idx(gp))
            ss.append(_wrap_idx(sp))
        packed.append((np.vstack(gs), np.vstack(ss)))
    return widths, GL, packed


def _to_bf16(x):
    import ml_dtypes
    return np.asarray(x, dtype=ml_dtypes.bfloat16)


def _patch_tile_drain():
    """This walrus build rejects CTRL instructions with >2 sync waits; split
    the TileContext kernel-tail drain's waits into single-wait nops."""
    import concourse.mybir as mybir
    from concourse.tile import TileContext, ScopedClock
    if getattr(TileContext, "_drain_patched", False):
        return

    def _drain_and_barrier(self, tick_clock, wait_clock):
        probe = self.nc.sync.nop()
        wait_clock.add_sem_waits(probe.ins,
                                 ScopedClock({None: tick_clock.global_clock}))
        si = probe.ins.sync_info
        waits = list(si.on_wait) if si is not None else []
        if si is not None:
            si.on_wait = waits[:1]
        for w in waits[1:]:
            n = self.nc.sync.nop()
            if n.ins.sync_info is None:
                n.ins.sync_info = mybir.SyncInfo(on_wait=[w], on_update=[])
            else:
                n.ins.sync_info.on_wait = [w]
        self.nc.sync.drain()
        self.nc.all_engine_barrier()
        assert self.sems is not None
        popped = self.nc._tile_sem_poison_stack.pop()
        assert popped is self._sem_poison
        self.nc.clear_and_free_semaphores(list(self.sems.allocated().values()))
        self.nc.all_engine_barrier()

    TileContext._drain_and_barrier = _drain_and_barrier
    TileContext._drain_patched = True


def _build_program(GL, GLA, pw, pwa):
    import os
    PHASES = int(os.environ.get("KDBG_PHASES", "7"))
    NOSCAT = int(os.environ.get("KDBG_NOSCAT", "0"))
    NOGATH = int(os.environ.get("KDBG_NOGATH", "0"))
    REPEAT = int(os.environ.get("KDBG_REPEAT", "1"))
    import concourse.bacc as bacc
    import concourse.mybir as mybir
    from concourse.tile import TileContext
    from concourse import library_config
    _patch_tile_drain()

    bf16 = mybir.dt.bfloat16
    f32 = mybir.dt.float32
    i16 = mybir.dt.int16
    RELU = mybir.ActivationFunctionType.Relu

    nc = bacc.Bacc("TRN2", target_bir_lowering=False, debug=False,
                   num_devices=NC)

    f_iniT_sh = nc.dram_tensor("f_iniT_sh", [F, BS], bf16, kind="ExternalInput")
    w_i = nc.dram_tensor("w_i", [F, DP], bf16, kind="ExternalInput")
    w_h = nc.dram_tensor("w_h", [DP, DP], bf16, kind="ExternalInput")
    w_oa = nc.dram_tensor("w_oa", [AF + 1, D], bf16, kind="ExternalInput")
    w_om = nc.dram_tensor("w_om", [DP, D], bf16, kind="ExternalInput")
    atom_fT = nc.dram_tensor("atom_fT", [AF + 1, AS], bf16, kind="ExternalInput")
    seg = nc.dram_tensor("seg", [128, 4], f32, kind="ExternalInput")
    g_idx_b = nc.dram_tensor("g_idx_b", [8 * 128, GL // 16], i16, kind="ExternalInput")
    s_idx_b = nc.dram_tensor("s_idx_b", [8 * 128, GL // 16], i16, kind="ExternalInput")
    g_idx_a = nc.dram_tensor("g_idx_a", [4 * 128, GLA // 16], i16, kind="ExternalInput")
    s_idx_a = nc.dram_tensor("s_idx_a", [4 * 128, GLA // 16], i16, kind="ExternalInput")

    mols = nc.dram_tensor("mols", [MS, D], f32, kind="ExternalOutput")

    m0_q = [nc.dram_tensor(f"m0_q{w}", [WIN, DP], bf16, kind="Internal",
                           addr_space="Shared") for w in range(NW)]
    m0_rows = nc.dram_tensor("m0_rows", [BS, DP], bf16, kind="Internal")
    ACC_R = BS + 128
    ACC_A = AS + 128
    m1_acc = nc.dram_tensor("m1_acc", [ACC_R, DP], bf16, kind="Internal")
    m2_acc = nc.dram_tensor("m2_acc", [ACC_R, DP], bf16, kind="Internal")
    ms_acc = nc.dram_tensor("ms_acc", [ACC_A, DP], bf16, kind="Internal")
    m1_q = [nc.dram_tensor(f"m1_q{w}", [WIN, DP], bf16, kind="Internal",
                           addr_space="Shared") for w in range(NW)]
    h2_rows = nc.dram_tensor("h2_rows", [BS, DP], bf16, kind="Internal")
    h2_q = [nc.dram_tensor(f"h2_q{w}", [WIN, DP], bf16, kind="Internal",
                           addr_space="Shared") for w in range(NW)]

    rg = [list(range(NC))]

    with TileContext(nc, num_cores=NC) as tc:
        with tc.tile_pool(name="const", bufs=1) as cpool:
            nc.gpsimd.load_library(library_config.mlp)

            wi_a = cpool.tile([128, DP], bf16)
            wi_b = cpool.tile([F - 128, DP], bf16)
            nc.sync.dma_start(wi_a[:], w_i[0:128, :])
            nc.sync.dma_start(wi_b[:], w_i[128:F, :])

            for _rep in range(REPEAT):
                # zero the scatter accumulators (covers trash rows)
                with tc.tile_pool(name="zpool", bufs=1) as zpool:
                    zt = zpool.tile([128, 24 * DP], bf16)
                    nc.vector.memset(zt[:], 0.0)
                    for acc, rows in ((m1_acc, ACC_R), (m2_acc, ACC_R),
                                      (ms_acc, ACC_A)):
                        r0 = 0
                        while r0 < rows:
                            n = min(rows - r0, 24 * 128)
                            nc.sync.dma_start(
                                acc[r0:r0 + n, :].rearrange("(s p) d -> p s d", p=128),
                                zt[:, : (n // 128) * DP].rearrange(
                                    "p (s d) -> p s d", d=DP))
                            r0 += n

                # ---- phase 0: m0 = relu(f_ini @ W_i), replicated over cores ----
                with (
                    tc.tile_pool(name="p0", bufs=3) as pool,
                    tc.tile_pool(name="p0ps", bufs=8, space="PSUM") as pspool,
                ):
                    for s0 in range(0, BS, SL):
                        fa = pool.tile([128, SL], bf16, tag="fa")
                        fb = pool.tile([F - 128, SL], bf16, tag="fb")
                        nc.sync.dma_start(fa[:], f_iniT_sh[0:128, s0:s0 + SL])
                        nc.sync.dma_start(fb[:], f_iniT_sh[128:F, s0:s0 + SL])
                        stage = pool.tile([128, SL // 128, DP], bf16, tag="st")
                        for t in range(SL // 128):
                            ps = pspool.tile([128, DP], f32)
                            nc.tensor.matmul(ps[:], fa[:, t * 128:(t + 1) * 128],
                                             wi_a[:], start=True, stop=False)
                            nc.tensor.matmul(ps[:], fb[:, t * 128:(t + 1) * 128],
                                             wi_b[:], start=False, stop=True)
                            if t % 2 == 0:
                                nc.scalar.activation(stage[:, t, :], ps[:], RELU)
                            else:
                                nc.vector.tensor_scalar_max(stage[:, t, :], ps[:], 0.0)
                        nc.sync.dma_start(
                            m0_rows[s0:s0 + SL, :].rearrange("(s p) d -> p s d", p=128),
                            stage[:])

                # ---- gather-sum round ----
                def gsum_round(tables, acc, g_idx, s_idx, widths, GLr, nchunk):
                    with tc.tile_pool(name="rnd", bufs=2) as pool:
                        for k in range(nchunk):
                            git = pool.tile([128, GLr // 16], i16, tag="git")
                            sit = pool.tile([128, GLr // 16], i16, tag="sit")
                            nc.sync.dma_start(git[:], g_idx[k * 128:(k + 1) * 128, :])
                            nc.sync.dma_start(sit[:], s_idx[k * 128:(k + 1) * 128, :])
                            buf = pool.tile([128, GLr // 128, DP], bf16, tag="buf")
                            if not NOGATH:
                                off = 0
                                for w in range(NW):
                                    pwv = int(widths[k][w])
                                    if pwv == 0:
                                        continue
                                    nc.gpsimd.dma_gather(
                                        buf[:, off // 128:(off + pwv) // 128, :],
                                        tables[w][:, :],
                                        git[:, off // 16:(off + pwv) // 16],
                                        pwv, pwv, DP, single_packet=False)
                                    off += pwv
                            else:
                                nc.vector.memset(buf[:], 0.0)
                            if not NOSCAT:
                                s0_ = 0
                                while s0_ < GLr:
                                    n_ = min(4096, GLr - s0_)
                                    nc.gpsimd.dma_scatter_add(
                                        acc[:, :],
                                        buf[:, s0_ // 128:(s0_ + n_) // 128, :],
                                        sit[:, s0_ // 16:(s0_ + n_) // 16],
                                        n_, n_, DP, single_packet=False)
                                    s0_ += n_

                # ---- round 1, AllGather, round 2 ----
                for w in range(NW):
                    nc.gpsimd.collective_compute(
                        "AllGather", mybir.AluOpType.bypass,
                        ins=[m0_rows[w * QS:(w + 1) * QS, :]],
                        outs=[m0_q[w][:, :]], replica_groups=rg)
                if PHASES >= 1:
                    gsum_round(m0_q, m1_acc, g_idx_b, s_idx_b, pw, GL, 8)
                if PHASES >= 2:
                    for w in range(NW):
                        nc.gpsimd.collective_compute(
                            "AllGather", mybir.AluOpType.bypass,
                            ins=[m1_acc[w * QS:(w + 1) * QS, :]],
                            outs=[m1_q[w][:, :]], replica_groups=rg)
                if PHASES >= 3:
                    gsum_round(m1_q, m2_acc, g_idx_b, s_idx_b, pw, GL, 8)
                if PHASES < 7:
                    with tc.tile_pool(name="dbg", bufs=1) as dpool:
                        dt_ = dpool.tile([128, 2 * D], f32)
                        nc.vector.memset(dt_[:], 0.0)
                        nc.sync.dma_start(
                            mols[:, :].rearrange("(s p) d -> p s d", p=128), 
                            dt_[:].rearrange("p (s d) -> p s d", d=D))
                if PHASES >= 4:

                    # ---- h2 = relu([f_ini_shard | m2] @ [W_i | W_h]) ----
                    with tc.tile_pool(name="m2t", bufs=1) as mpool:
                        m2T = [mpool.tile([128, BS], bf16, tag=f"m2T{i}", name=f"m2T{i}") for i in range(3)]
                        for ft in range(3):
                            nc.sync.dma_start_transpose(
                                m2T[ft][:], m2_acc[0:BS, ft * 128:(ft + 1) * 128])
                        wh_t = [mpool.tile([128, DP], bf16, tag=f"wh{i}", name=f"wh{i}") for i in range(3)]
                        for kt in range(3):
                            nc.sync.dma_start(wh_t[kt][:], w_h[kt * 128:(kt + 1) * 128, :])
                        with (
                            tc.tile_pool(name="h2", bufs=3) as pool,
                            tc.tile_pool(name="h2ps", bufs=8, space="PSUM") as pspool,
                        ):
                            for s0 in range(0, BS, SL):
                                fa = pool.tile([128, SL], bf16, tag="fa")
                                fb = pool.tile([F - 128, SL], bf16, tag="fb")
                                nc.sync.dma_start(fa[:], f_iniT_sh[0:128, s0:s0 + SL])
                                nc.sync.dma_start(fb[:], f_iniT_sh[128:F, s0:s0 + SL])
                                stage = pool.tile([128, SL // 128, DP], bf16, tag="st")
                                for t in range(SL // 128):
                                    b0 = s0 + t * 128
                                    ps = pspool.tile([128, DP], f32)
                                    nc.tensor.matmul(ps[:], fa[:, t * 128:(t + 1) * 128],
                                                     wi_a[:], start=True, stop=False)
                                    nc.tensor.matmul(ps[:], fb[:, t * 128:(t + 1) * 128],
                                                     wi_b[:], start=False, stop=False)
                                    for kt in range(3):
                                        nc.tensor.matmul(ps[:], m2T[kt][:, b0:b0 + 128],
                                                         wh_t[kt][:], start=False,
                                                         stop=(kt == 2))
                                    if t % 2 == 0:
                                        nc.scalar.activation(stage[:, t, :], ps[:], RELU)
                                    else:
                                        nc.vector.tensor_scalar_max(stage[:, t, :], ps[:], 0.0)
                                nc.sync.dma_start(
                                    h2_rows[s0:s0 + SL, :].rearrange("(s p) d -> p s d", p=128),
                                    stage[:])
                    if PHASES >= 5:
                        for w in range(NW):
                            nc.gpsimd.collective_compute(
                                "AllGather", mybir.AluOpType.bypass,
                                ins=[h2_rows[w * QS:(w + 1) * QS, :]],
                                outs=[h2_q[w][:, :]], replica_groups=rg)

                    # ---- atom round ----
                    if PHASES >= 6:
                        gsum_round(h2_q, ms_acc, g_idx_a, s_idx_a, pwa, GLA, 4)

                    # ---- atoms_h ----
                    with tc.tile_pool(name="atom", bufs=1) as apool:
                        msT = [apool.tile([128, AS], bf16, tag=f"msT{i}", name=f"msT{i}") for i in range(3)]
                        for ft in range(3):
                            nc.sync.dma_start_transpose(
                                msT[ft][:], ms_acc[0:AS, ft * 128:(ft + 1) * 128])
                        afa = apool.tile([128, AS], bf16)
                        afb = apool.tile([AF + 1 - 128, AS], bf16)
                        nc.sync.dma_start(afa[:], atom_fT[0:128, :])
                        nc.sync.dma_start(afb[:], atom_fT[128:AF + 1, :])
                        woa_a = apool.tile([128, D], bf16)
                        woa_b = apool.tile([AF + 1 - 128, D], bf16)
                        nc.sync.dma_start(woa_a[:], w_oa[0:128, :])
                        nc.sync.dma_start(woa_b[:], w_oa[128:AF + 1, :])
                        wom_t = [apool.tile([128, D], bf16, tag=f"wom{i}", name=f"wom{i}") for i in range(3)]
                        for kt in range(3):
                            nc.sync.dma_start(wom_t[kt][:], w_om[kt * 128:(kt + 1) * 128, :])
                        seg_t = apool.tile([128, 4], f32)
                        nc.sync.dma_start(seg_t[:], seg[:])
                        with (
                            tc.tile_pool(name="ah", bufs=4) as pool,
                            tc.tile_pool(name="ahps", bufs=4, space="PSUM") as pspool,
                        ):
                            for at in range(AS // 128):
                                a0 = at * 128
                                ps = pspool.tile([128, D], f32, tag="ps")
                                nc.tensor.matmul(ps[:], afa[:, a0:a0 + 128], woa_a[:],
                                                 start=True, stop=False)
                                nc.tensor.matmul(ps[:], afb[:, a0:a0 + 128], woa_b[:],
                                                 start=False, stop=False)
                                for kt in range(3):
                                    nc.tensor.matmul(ps[:], msT[kt][:, a0:a0 + 128],
                                                     wom_t[kt][:], start=False,
                                                     stop=(kt == 2))
                                ah = pool.tile([128, D], f32, tag="ah")
                                nc.vector.tensor_scalar_max(ah[:], ps[:], 0.0)
                                mp = pspool.tile([4, D], f32, tag="mp")
                                nc.tensor.matmul(mp[:], seg_t[:], ah[:],
                                                 start=True, stop=True)
                                msml = pool.tile([4, D], f32, tag="msml")
                                nc.vector.tensor_copy(msml[:], mp[:])
                                nc.sync.dma_start(mols[at * 4:(at + 1) * 4, :], msml[:])



    nc.compile()
    return nc


def _get_program(GL, GLA, pw, pwa):
    import os
    key = (GL, GLA, tuple(map(tuple, pw)), tuple(map(tuple, pwa)),
           os.environ.get("KDBG_PHASES", "7"), os.environ.get("KDBG_REPEAT", "1"))
    if key not in _CACHE:
        _CACHE[key] = _build_program(GL, GLA, pw, pwa)
    return _CACHE[key]


def kernel(atom_features, f_ini_atoms_bonds, global_features, W_i, W_h, W_o, b_o,
           atom_to_incoming_bonds, mapping, atom_to_mol):
    from concourse import bass_utils

    atom_features = np.asarray(atom_features, np.float32)
    f_ini = np.asarray(f_ini_atoms_bonds, np.float32)
    global_features = np.asarray(global_features, np.float32)
    W_i_np = np.asarray(W_i, np.float32)
    W_h_np = np.asarray(W_h, np.float32)
    W_o_np = np.asarray(W_o, np.float32)
    b_o_np = np.asarray(b_o, np.float32)
    a2b = np.asarray(atom_to_incoming_bonds, np.int32)
    mp_idx = np.asarray(mapping, np.int32)

    f_iniT = np.ascontiguousarray(f_ini.T)
    wi_pad = np.zeros((F, DP), np.float32)
    wi_pad[:, :D] = W_i_np
    wh_pad = np.zeros((DP, DP), np.float32)
    wh_pad[:D, :D] = W_h_np
    woa = np.zeros((AF + 1, D), np.float32)
    woa[:AF] = W_o_np[:AF]
    woa[AF] = b_o_np
    wom = np.zeros((DP, D), np.float32)
    wom[:D] = W_o_np[AF:]
    atom_fT_full = np.zeros((AF + 1, A), np.float32)
    atom_fT_full[:AF] = atom_features.T
    atom_fT_full[AF] = 1.0
    seg = np.zeros((128, 4), np.float32)
    for q in range(4):
        seg[q * 32:(q + 1) * 32, q] = 1.0 / APM

    bond_plans = [_plan_round(mp_idx[c * BS:(c + 1) * BS], BS, 2)
                  for c in range(NC)]
    atom_plans = [_plan_round(a2b[c * AS:(c + 1) * AS], AS, 1)
                  for c in range(NC)]
    pw, GL, bond_packed = _pack_round(bond_plans, BS)
    pwa, GLA, atom_packed = _pack_round(atom_plans, AS)

    prog = _get_program(GL, GLA, pw.tolist(), pwa.tolist())

    f_iniT_bf = _to_bf16(f_iniT)
    shared = {
        "w_i": _to_bf16(wi_pad),
        "w_h": _to_bf16(wh_pad),
        "w_oa": _to_bf16(woa),
        "w_om": _to_bf16(wom),
        "seg": seg,
    }
    in_maps = []
    for c in range(NC):
        m = dict(shared)
        m["f_iniT_sh"] = np.ascontiguousarray(
            f_iniT_bf[:, c * BS:(c + 1) * BS])
        m["atom_fT"] = _to_bf16(atom_fT_full[:, c * AS:(c + 1) * AS])
        m["g_idx_b"], m["s_idx_b"] = bond_packed[c]
        m["g_idx_a"], m["s_idx_a"] = atom_packed[c]
        in_maps.append(m)

    global LAST_RESULTS
    res = bass_utils.run_bass_kernel_spmd(prog, in_maps, core_ids=list(range(NC)))
    LAST_RESULTS = res
    mols = np.concatenate([res.results[c]["mols"] for c in range(NC)], 0)
    return np.concatenate([mols, global_features], 1).astype(np.float32)

